# revision 73
# baseline (speedup 1.0000x reference)
"""CropToBBox (crop_and_resize to 224x224 with bbox preprocessing) on 8 trn2 cores.

v5 strategy:
  - Host computes the separable bilinear hat-weight matrices (fp32 coords ->
    bf16) and slices each image's source window into dense bf16 uploads; no
    on-device weight build, no SWDGE gathers.
  - Separable resize as two bf16 matmul stages (full rate at 224 moving dim).
    Slots with Hc < Wc contract columns first (x-first) -- fewer stage-2
    chains and smaller intermediates.
  - Degenerate boxes (threshold < 0.5) crop a single pixel: their outputs
    are built by a gpsimd partition_broadcast + stride-0 replicating copies,
    no PE work.
  - PSUM: pv [128,768]x3 for stage 1, po [112,512]x2 for stage 2; per-third
    PSUM->SBUF copies (ACT/DVE balanced) and per-third output DMAs; the
    final slot drains per-group to shorten the kernel tail.
  - Uploads are spread across the SP/ACT HWDGE lanes and the pool SWDGE
    lane by deadline; output DMAs alternate SP/pool; ACT only ever gets
    DMAs issued at t=0 so its SEQ never blocks later copies.
  - PE warmup matmuls on junk data during the upload phase ramp the tensor
    engine to full clock before real work arrives.
"""

import numpy as np
import ml_dtypes

N_FULL = 64
H = W = 512
C = 3
CH = CW = 224
N_CORES = 8
PER_CORE = N_FULL // N_CORES
FACTOR = 1.2
BAD = np.float32(-1e5)
BF16 = ml_dtypes.bfloat16

_CACHE = {}


def _host_coords(threshold, bboxes):
    """Replicate process_bbox + crop_and_resize coordinate math in fp32.

    Returns ys, xs [64, 224] with BAD at invalid (out-of-range) positions.
    """
    f = np.float32
    th = np.asarray(threshold, f)
    bb = np.asarray(bboxes, f)
    default = np.array([0.0, 1.0, 0.0, 1.0], f)
    filt = np.where(th < f(0.5), default, bb).astype(f)
    x1, y1, x2, y2 = filt[:, 0], filt[:, 1], filt[:, 2], filt[:, 3]

    def resize_side(small, large):
        side = (large - small).astype(f)
        new_side = (side * f(FACTOR)).astype(f)
        center = ((small + large) / f(2)).astype(f)
        half = (new_side / f(2)).astype(f)
        new_min = np.clip((center - half).astype(f), f(0), f(1)).astype(f)
        new_max = np.clip((center + half).astype(f), f(0), f(1)).astype(f)
        return new_min, new_max

    nx1, nx2 = resize_side(x1, x2)
    ny1, ny2 = resize_side(y1, y2)
    # reference: boxes = stack([nx1, ny1, nx2, ny2]); crop uses [y1,x1,y2,x2]
    by1, bx1, by2, bx2 = nx1, ny1, nx2, ny2

    idx = np.arange(CH, dtype=f)
    ys = (by1[:, None] * f(H - 1)).astype(f) + (
        idx[None, :] * (((by2 - by1) * f(H - 1)).astype(f) / f(CH - 1)).astype(f)[:, None]
    ).astype(f)
    ys = ys.astype(f)
    xs = (bx1[:, None] * f(W - 1)).astype(f) + (
        idx[None, :] * (((bx2 - bx1) * f(W - 1)).astype(f) / f(CW - 1)).astype(f)[:, None]
    ).astype(f)
    xs = xs.astype(f)

    ys = np.where((ys >= f(0)) & (ys <= f(H - 1)), ys, BAD).astype(f)
    xs = np.where((xs >= f(0)) & (xs <= f(W - 1)), xs, BAD).astype(f)
    return ys, xs


def _windows(ys, xs):
    """Per image: row window (r0, S) and 64px col-block window (cb0, K)."""
    out = []
    for n in range(N_FULL):
        yv = ys[n][ys[n] > -1e4]
        xv = xs[n][xs[n] > -1e4]
        if yv.size == 0 or xv.size == 0:
            # fully out-of-range crop: output is all zeros (constant)
            out.append((0, 1, 0, 1, 2))
            continue
        r0 = int(np.floor(yv.min())); r1 = int(np.ceil(yv.max()))
        r0 = max(0, min(r0, H - 1)); r1 = max(r0, min(r1, H - 1))
        c0 = int(np.floor(xv.min())); c1 = int(np.ceil(xv.max()))
        c0 = max(0, min(c0, W - 1)); c1 = max(c0, min(c1, W - 1))
        cb0 = c0 // 64
        K = c1 // 64 - cb0 + 1
        # cint=1: every output pixel samples the same integer source pixel
        # (degenerate thresholded boxes) -> constant output per channel
        cint = int(yv.min() == yv.max() and xv.min() == xv.max()
                   and float(yv[0]).is_integer() and float(xv[0]).is_integer()
                   and yv.size == CH and xv.size == CH)
        out.append((r0, r1 - r0 + 1, cb0, K, cint))
    return out


def _hcwc(S, K):
    return -(-S // 128), -(-(64 * K) // 128)


def _slot_cost(S, K):
    """Per-slot ns cost estimate: PE matmuls + img upload + psum copies."""
    Hc, Wc = _hcwc(S, K)
    pe = 93.0 * (3 * Hc * Wc + 6 * Wc)
    dma = S * K * 1.07  # bf16 window upload, 384B/row-block at 360GB/s
    cp = (672.0 * Wc + 1344.0) * 0.9
    return pe + 0.5 * dma + 0.5 * cp


def _plan(wins):
    """Assign 64 images to 8 slots x 8 cores; trivial slots first.

    Returns perm[core][slot] -> image index, and the signature tuple of
    per-slot (S, K).
    """
    area = np.array([_slot_cost(w[1], w[3]) for w in wins])
    order = np.argsort(-area, kind="stable")
    groups = [list(order[j * 8:(j + 1) * 8]) for j in range(8)]

    def gcost(g):
        S = max(wins[i][1] for i in g)
        K = max(wins[i][3] for i in g)
        return _slot_cost(S, K)

    for _ in range(6):
        improved = False
        for a in range(8):
            for b in range(a + 1, 8):
                base = gcost(groups[a]) + gcost(groups[b])
                best = None
                for ia in range(8):
                    for ib in range(8):
                        ga = groups[a][:]; gb = groups[b][:]
                        ga[ia], gb[ib] = gb[ib], ga[ia]
                        c = gcost(ga) + gcost(gb)
                        if c < base - 1e-9:
                            base = c; best = (ia, ib)
                if best is not None:
                    ia, ib = best
                    groups[a][ia], groups[b][ib] = groups[b][ib], groups[a][ia]
                    improved = True
        if not improved:
            break

    def is_trivial(g):
        return all(wins[i][1] == 1 and wins[i][3] == 1 and wins[i][4]
                   for i in g)

    # second pass: move loner images under another slot's (S, K) umbrella
    # when the swap makes a group trivial or strictly cheaper
    for _ in range(4):
        improved = False
        for a in range(8):
            for b in range(8):
                if a == b:
                    continue
                base = gcost(groups[a]) + gcost(groups[b])
                for ia in range(8):
                    for ib in range(8):
                        ga = groups[a][:]; gb = groups[b][:]
                        ga[ia], gb[ib] = gb[ib], ga[ia]
                        c = gcost(ga) + gcost(gb)
                        if c < base - 1e-9:
                            groups[a], groups[b] = ga, gb
                            base = c
                            improved = True
        if not improved:
            break

    # trivial slots first (cheap, warm the PE), then ascending cost
    groups.sort(key=lambda g: (not is_trivial(g), gcost(g)))
    sig = []
    perm = [[0] * 8 for _ in range(N_CORES)]
    for j, g in enumerate(groups):
        S = max(wins[i][1] for i in g)
        K = max(wins[i][3] for i in g)
        sig.append((S, K, int(is_trivial(g))))
        for c, img in enumerate(g):
            perm[c][j] = img
    return perm, tuple(sig)


def _layout(sig):
    """Column layouts for the xw (image windows) and wt (weights) uploads.

    Returns dict with ntriv, per-slot geometry, offsets, totals.
    """
    ntriv = 0
    for ent in sig:
        if ent[2]:
            ntriv += 1
        else:
            break
    slots = []
    off_f = 0
    # weights: [px ntriv*3 on partition 0][real slot chunks 224 each]
    off_w = ntriv * 3
    for j, (S, K, _tv) in enumerate(sig):
        Hc, Wc = _hcwc(S, K)
        if j < ntriv:
            slots.append(dict(S=S, K=K, Hc=Hc, Wc=Wc, triv=True,
                              px=j * 3, f=None))
        else:
            # x-first: contract cols in stage 1 -- fewer stage-2 chains
            # when Hc < Wc, and a smaller upload on ties (S*3 vs K*192)
            xf = Hc < Wc or (Hc == Wc and S * 3 < K * 192)
            elemW = (S * 3) if xf else (K * 192)
            nch = Wc if xf else Hc  # image chunks along the contract dim
            slots.append(dict(S=S, K=K, Hc=Hc, Wc=Wc, triv=False, xf=xf,
                              nch=nch, elemW=elemW, f=off_f, wy=off_w,
                              wx=off_w + Hc * CH))
            off_f += nch * elemW
            off_w += (Hc + Wc) * CH
    return dict(ntriv=ntriv, slots=slots, totf=max(off_f, 1), totw=off_w)


def _host_arrays(images, ys, xs, wins, perm, sig):
    """Build per-core input dicts: xw (bf16 windows), wt (bf16 weights)."""
    f = np.float32
    lay = _layout(sig)
    ntriv, slots, totf, totw = (lay["ntriv"], lay["slots"], lay["totf"],
                                lay["totw"])
    p = np.arange(128, dtype=f)[:, None]
    iw = images.view()  # [64, H, W, C] fp32

    in_maps = []
    for core in range(N_CORES):
        xw = np.zeros((128, totf), BF16)
        wt = np.zeros((128, totw), BF16)
        for j, sl in enumerate(slots):
            n = perm[core][j]
            r0, Sn, cb0n, Kn, cint = wins[n]
            S, K = sl["S"], sl["K"]
            cb0 = min(cb0n, 8 - K)
            ysn = ys[n] - f(r0)        # [224], BAD stays hugely negative
            xsn = xs[n] - f(64 * cb0)
            if sl["triv"]:
                # constant crop: every output pixel is one source pixel
                if cint == 1:
                    wi = int(xs[n][0])
                    wt[0, sl["px"]:sl["px"] + 3] = \
                        iw[n, r0, wi, :].astype(BF16)
                # cint==2 (all out of range) -> output 0; px stays 0
            else:
                Hc, Wc, elemW = sl["Hc"], sl["Wc"], sl["elemW"]
                # window rows [r0, r0+S) x cols [64cb0, 64(cb0+K)) in bf16;
                # rows past the image edge are zero (their hat weight is 0)
                rS = min(S, H - r0)
                win = np.zeros((S, 64 * K, C), np.float32)
                win[0:rS] = iw[n, r0:r0 + rS, 64 * cb0:64 * (cb0 + K), :]
                if sl["xf"]:
                    # transposed: chunk rows = source cols, line = (h, c)
                    wf = win.transpose(1, 0, 2).reshape(64 * K, elemW)
                    wf = wf.astype(BF16)
                    for k in range(Wc):
                        rows = min(128, 64 * K - 128 * k)
                        xw[0:rows,
                           sl["f"] + k * elemW: sl["f"] + (k + 1) * elemW] = \
                            wf[128 * k:128 * k + rows]
                else:
                    wf = win.reshape(S, elemW).astype(BF16)
                    for k in range(Hc):
                        rows = min(128, S - 128 * k)
                        xw[0:rows,
                           sl["f"] + k * elemW: sl["f"] + (k + 1) * elemW] = \
                            wf[128 * k:128 * k + rows]
                for k in range(Hc):
                    wy = np.maximum(0.0, 1.0 - np.abs(ysn[None, :] - (p + 128 * k)))
                    wt[:, sl["wy"] + k * CH: sl["wy"] + (k + 1) * CH] = wy.astype(BF16)
                for k in range(Wc):
                    wx = np.maximum(0.0, 1.0 - np.abs(xsn[None, :] - (p + 128 * k)))
                    wt[:, sl["wx"] + k * CH: sl["wx"] + (k + 1) * CH] = wx.astype(BF16)
        in_maps.append({"xw": xw, "wt": wt})
    return in_maps


def _build_nc(sig):
    from concourse import bacc, tile
    import concourse.mybir as mybir

    dt = mybir.dt
    F32 = dt.float32
    BF = dt.bfloat16
    AF = mybir.ActivationFunctionType

    lay = _layout(sig)
    ntriv, slots, totf, totw = (lay["ntriv"], lay["slots"], lay["totf"],
                                lay["totw"])
    real = [j for j in range(8) if not slots[j]["triv"]]

    nc = bacc.Bacc(None, target_bir_lowering=False, num_swdge_queues=4)
    xw_d = nc.declare_dram_parameter("xw", [128, totf], BF, isOutput=False)
    wt_d = nc.declare_dram_parameter("wt", [128, totw], BF, isOutput=False)
    out_d = nc.declare_dram_parameter("out", [PER_CORE, C, CH, CW], F32,
                                      isOutput=True)

    # engine-busy bookkeeping for greedy copy placement (ACT vs DVE)
    load = {"scalar": 0.0, "vector": 0.0}

    def copy_cost(eng, free):
        if eng == "scalar":
            return free * 0.833 + 287.0
        return free * 1.042 + 250.0

    def emit_copy(nc, dst, src, free, eng=None):
        e = eng or min(("scalar", "vector"), key=lambda x: load[x] + copy_cost(x, free))
        if e == "scalar":
            load[e] += copy_cost(e, free)
            nc.scalar.activation(dst, src, AF.Copy, bias=0.0, scale=1.0)
        elif e == "gpsimd":
            # SBUF->SBUF only (Pool cannot read PSUM)
            nc.gpsimd.tensor_copy(dst, src)
        else:
            load[e] += copy_cost(e, free)
            nc.vector.tensor_copy(dst, src)
        return e

    with tile.TileContext(nc) as tc:
        with (
            tc.tile_pool(name="const", bufs=1) as cpool,
            tc.tile_pool(name="vt", bufs=3) as vpool,
            tc.tile_pool(name="osb", bufs=3) as opool,
            tc.tile_pool(name="tosb", bufs=2) as topool,
            tc.tile_pool(name="pv", bufs=2, space="PSUM") as pvpool,
            tc.tile_pool(name="po", bufs=4, space="PSUM") as popool,
        ):
            # batched upload tiles: group A (trivial weights + first two real
            # slots) lands first on the SP lane; the rest go via SWDGE so no
            # upload ever sits on the ACT SEQ (ACT is reserved for copies).
            # Tile granularity matches the dependency groups.
            wt_cut0 = ntriv * 3 + ntriv * CH
            lane_ns = {"sync": 0.0, "scalar": 0.0, "gpsimd": 0.0}

            def lane_pick(ns, exclude=("scalar",)):
                e = min((k for k in lane_ns if k not in exclude),
                        key=lambda k: lane_ns[k])
                lane_ns[e] += ns
                return getattr(nc, e)

            # uploads (deadline-balanced; per-lane transfers serialize):
            #   sync:   wtA=[trivial px | s4 weights], xw4, xw6
            #   scalar: xw5 (issued at t=0 only; ACT stays free for copies)
            #   pool:   wtB (s5 w), wtC (s6+s7 w), xw7
            wt_tile, xw_tile = {}, {}
            r0 = real[0] if real else None
            r1 = real[1] if len(real) > 1 else None
            groupB = real[2:]
            e0 = (slots[r0]["wx"] + slots[r0]["Wc"] * CH) if real else wt_cut0
            wtA = cpool.tile([128, e0], BF, name="wtA") if e0 else None
            if wtA is not None:
                nc.sync.dma_start(out=wtA[:], in_=wt_d[:, 0:e0])
            if r0 is not None:
                wt_tile[r0] = (wtA, 0)
                sl = slots[r0]
                f0, f1 = sl["f"], sl["f"] + sl["nch"] * sl["elemW"]
                xw4 = cpool.tile([128, f1 - f0], BF, name="xw4")
                nc.sync.dma_start(out=xw4[:], in_=xw_d[:, f0:f1])
                xw_tile[r0] = (xw4, f0)
            if r1 is not None:
                b0 = slots[r1]["wy"]
                b1 = slots[r1]["wx"] + slots[r1]["Wc"] * CH
                wtB = cpool.tile([128, b1 - b0], BF, name="wtB")
                nc.gpsimd.dma_start(out=wtB[:], in_=wt_d[:, b0:b1])
                wt_tile[r1] = (wtB, b0)
                sl = slots[r1]
                f0, f1 = sl["f"], sl["f"] + sl["nch"] * sl["elemW"]
                xw5 = cpool.tile([128, f1 - f0], BF, name="xw5")
                nc.scalar.dma_start(out=xw5[:], in_=xw_d[:, f0:f1])
                xw_tile[r1] = (xw5, f0)
            if groupB:
                c0 = slots[groupB[0]]["wy"]
                wtC = cpool.tile([128, totw - c0], BF, name="wtC")
                nc.gpsimd.dma_start(out=wtC[:], in_=wt_d[:, c0:totw])
                for j in groupB:
                    wt_tile[j] = (wtC, c0)
            for bi, j in enumerate(groupB):
                sl = slots[j]
                f0, f1 = sl["f"], sl["f"] + sl["nch"] * sl["elemW"]
                t = cpool.tile([128, f1 - f0], BF, name=f"xw{j}")
                eng = nc.sync if bi % 2 == 0 else nc.gpsimd
                eng.dma_start(out=t[:], in_=xw_d[:, f0:f1])
                xw_tile[j] = (t, f0)

            def wt_ap(j, rows, lo, hi):
                t, base = wt_tile[j]
                return t[0:rows, lo - base:hi - base]

            # --- PE warmup + ACT table preload while uploads land ---
            junk = cpool.tile([1, 336], BF, name="junk")
            nc.vector.memset(junk[:], 0.0)
            jact = cpool.tile([1, 2], F32, name="jact")
            nc.vector.memset(jact[:], 0.0)
            nc.scalar.activation(jact[:, 1:2], jact[:, 0:1], AF.Copy,
                                 bias=0.0, scale=1.0)
            pw = pvpool.tile([128, 768], F32, tag="pv")
            for _ in range(8):
                nc.tensor.matmul(pw[0:112, 0:224], junk[:, 0:112],
                                 junk[:, 112:336], start=True, stop=True)

            # --- trivial slots: constant crops, no PE work ---
            # px values (one per image,channel) sit on partition 0 of wtA;
            # partition_broadcast sends them down 112 partitions, stride-0
            # copies replicate along j, and the out DMA (one per ic half)
            # replays the tosb row block for both output halves
            from concourse import library_config
            nc.gpsimd.load_library(library_config.mlp)
            tosbs = {}
            if ntriv:
                pb = cpool.tile([112, ntriv * 3], BF, name="pb")
                nc.gpsimd.partition_broadcast(pb[:], wtA[0:1, 0:ntriv * 3])

            def emit_tcopies():
                for base in range(0, ntriv, 2):
                    nimg = min(2, ntriv - base)
                    tosb = topool.tile([112, nimg * 3 * CH], F32,
                                       tag="tosb", name=f"tosb{base}")
                    tosbs[base] = tosb
                    src = pb[:, base * 3:(base + nimg) * 3].unsqueeze(2) \
                        .broadcast_to([112, nimg * 3, CH])
                    dst = tosb[:].rearrange("p (g x) -> p g x", x=CH)
                    emit_copy(nc, dst, src, nimg * 3 * CH,
                              eng="gpsimd" if base == 0 else None)

            def emit_touts():
                for base in sorted(tosbs):
                    nimg = min(2, ntriv - base)
                    for ic in range(2):
                        dst = out_d[base:base + nimg].rearrange(
                            "m c (ic p) x -> p (m c) ic x", ic=2)[:, :, ic]
                        src = tosbs[base][:].rearrange(
                            "p (g x) -> p g x", x=CH)
                        lane_pick(nimg * 836.0).dma_start(out=dst, in_=src)

            # --- real slots: stage 1 -> v copies -> (deferred) stage 2 ---
            def emit_stage1(j):
                # y-first: V[w, i] = sum_h img[h, w] wy[h, i]; x-first (when
                # Hc < Wc): V'[h, j] = sum_w imgT[w, h] wx[w, j].  The
                # stage-1 weights are wy (y-first) or wx (x-first); stage 2
                # contracts the other axis.
                sl = slots[j]
                xf, nch = sl["xf"], sl["nch"]
                Hc, Wc, Wpx, elemW = (sl["Hc"], sl["Wc"], 64 * sl["K"],
                                      sl["elemW"])
                nout = Hc if xf else Wc
                osize = sl["S"] if xf else Wpx
                w1 = sl["wx"] if xf else sl["wy"]
                xt, xbase = xw_tile[j]
                xv = xt[:, sl["f"] - xbase:sl["f"] - xbase + nch * elemW] \
                    .rearrange("p (k w c) -> p k w c", k=nch, c=C)
                vts = []
                for ok in range(nout):
                    oseg = min(128, osize - 128 * ok)
                    pv = pvpool.tile([128, 768], F32, tag="pv")
                    for ci in range(C):
                        dstp = pv[0:oseg, ci * 256:ci * 256 + 224]
                        for k in range(nch):
                            nc.tensor.matmul(
                                dstp,
                                xv[:, k, 128 * ok:128 * ok + oseg, ci],
                                wt_ap(j, 128, w1 + k * CH, w1 + (k + 1) * CH),
                                start=(k == 0), stop=(k == nch - 1))
                    v = vpool.tile([128, 3 * CH], BF, tag=f"v{ok}")
                    src = pv[0:oseg].rearrange("p (ci x) -> p ci x", ci=3)[:, :, 0:224]
                    emit_copy(nc, v[0:oseg].rearrange("p (ci x) -> p ci x", ci=3),
                              src, 3 * CH)
                    vts.append(v)
                return vts

            def emit_stage2(j, vts, last):
                sl = slots[j]
                Wc, Wpx = sl["Wc"], 64 * sl["K"]
                xf = sl["xf"]
                nacc = Wc if not xf else sl["Hc"]
                asize = Wpx if not xf else sl["S"]
                osb = opool.tile([112, 6 * CH], F32, tag="osb")
                for third in range(3):
                    fine = last and third >= 1
                    pos = []
                    for g in range(2):
                        # the last slot's final groups get their own psum
                        # tiles so a group's matmuls never wait on the
                        # previous group's drain copy (tile-granular deps)
                        if g == 0 or fine:
                            po = popool.tile([112, 512], F32, tag="po")
                            pos.append(po)
                        ci, ic = divmod(third * 2 + g, 2)
                        dstp = po[:, (0 if fine else g) * 256:
                                  (0 if fine else g) * 256 + 224]
                        for wk in range(nacc):
                            wseg = min(128, asize - 128 * wk)
                            v = vts[wk]
                            if xf:
                                lhs = wt_ap(j, wseg, sl["wy"] + wk * CH,
                                            sl["wy"] + (wk + 1) * CH)[
                                    :, ic * 112:ic * 112 + 112]
                                rhs = v[0:wseg, ci * CH:(ci + 1) * CH]
                            else:
                                lhs = v[0:wseg,
                                        ci * CH + ic * 112:ci * CH + ic * 112 + 112]
                                rhs = wt_ap(j, wseg, sl["wx"] + wk * CH,
                                            sl["wx"] + (wk + 1) * CH)
                            nc.tensor.matmul(
                                dstp, lhs, rhs,
                                start=(wk == 0), stop=(wk == nacc - 1))
                        if fine:
                            gx = third * 2 + g
                            emit_copy(nc, osb[:, gx * CH:(gx + 1) * CH],
                                      po[:, 0:224], CH,
                                      eng=("scalar", "vector")[gx % 2])
                            dsto = out_d[j].rearrange(
                                "c (ic p) x -> p (c ic) x", ic=2)[:, gx:gx + 1]
                            eng = (nc.gpsimd, nc.sync)[gx % 2]
                            eng.dma_start(
                                out=dsto,
                                in_=osb[:, gx * CH:(gx + 1) * CH].unsqueeze(1))
                    if not fine:
                        src = pos[0][:].rearrange("p (g x) -> p g x", g=2)[:, :, 0:224]
                        dst = osb[:, third * 2 * CH:(third + 1) * 2 * CH] \
                            .rearrange("p (g x) -> p g x", g=2)
                        emit_copy(nc, dst, src, 2 * CH)
                        dsto = out_d[j].rearrange(
                            "c (ic p) x -> p (c ic) x", ic=2)[:, third * 2:third * 2 + 2]
                        lane_pick(557.0).dma_start(
                            out=dsto,
                            in_=osb[:, third * 2 * CH:(third + 1) * 2 * CH]
                            .rearrange("p (g x) -> p g x", g=2))

            # schedule: trivial copies+outs run on ACT/DVE/DMA lanes only,
            # interleaved after the first real stage 1; the usual
            # one-slot-deferred stage-2 pipeline follows
            vcache = {}
            if real:
                j0 = real[0]
                vcache[j0] = emit_stage1(j0)
            pend = real[0:1]
            for idx, j in enumerate(real[1:]):
                vcache[j] = emit_stage1(j)
                if idx == 1:
                    # trivial copies go behind the first three slots' v copies
                    emit_tcopies()
                    emit_touts()
                for p in pend:
                    emit_stage2(p, vcache.pop(p), last=False)
                pend = [j]
            for p in pend:
                emit_stage2(p, vcache.pop(p), last=True)
    nc.finalize()
    nc._engine_load_estimate = dict(load)
    return nc


def _get_nc(sig):
    key = ("nc", sig)
    if key not in _CACHE:
        _CACHE[key] = _build_nc(sig)
    return _CACHE[key]


def _ensure_device_platform():
    import jax
    try:
        if len([d for d in jax.devices() if d.platform != "cpu"]) >= N_CORES:
            return
    except Exception:
        pass
    import os
    os.environ.pop("JAX_PLATFORMS", None)
    try:
        jax.config.update("jax_platforms", None)
    except Exception:
        pass
    for clear in ("clear_backends",):
        try:
            getattr(jax, clear)()
            break
        except Exception:
            pass


def prepare(threshold, bboxes, images):
    """Host-side planning shared by kernel() and the sim test."""
    ys, xs = _host_coords(threshold, bboxes)
    wins = _windows(ys, xs)
    perm, sig = _plan(wins)
    images = np.ascontiguousarray(np.asarray(images, np.float32))
    in_maps = _host_arrays(images, ys, xs, wins, perm, sig)
    return in_maps, perm, sig


def assemble(results, perm):
    """results[core]["out"] [8, 3, 224, 224] -> full [64, 224, 224, 3]."""
    full = np.empty((N_FULL, CH, CW, C), np.float32)
    for core in range(N_CORES):
        o = np.asarray(results[core]["out"])
        o = np.transpose(o, (0, 2, 3, 1))
        for j in range(8):
            full[perm[core][j]] = o[j]
    return full


def kernel(threshold, bboxes, images):
    from concourse.bass_utils import run_bass_kernel_spmd

    _ensure_device_platform()
    in_maps, perm, sig = prepare(threshold, bboxes, images)
    nc = _get_nc(sig)
    _CACHE["nc"] = nc

    import os
    trace = bool(os.environ.get("CROP_TRACE"))
    if trace:
        try:
            import antenv.axon_hooks  # noqa: F401
        except ImportError:
            trace = False
    res = run_bass_kernel_spmd(nc, in_maps, list(range(N_CORES)), trace=trace)
    _CACHE["last_res"] = res
    return assemble(res.results, perm).astype(np.float32)
